# revision 13
# baseline (speedup 1.0000x reference)
"""KVCache prefill_draft eviction kernel for Trainium2 (8 NeuronCores).

Semantics (from the reference):
  - gather paged cache [1024,2,16,16,128] into per-seq linear [B=4, 2, L=4096, H=16, D=128]
  - sliding-window shift: new position l takes old position l+528 (l<3568),
    new tokens (3568<=l<4080), old position l (l>=4080)
  - keys re-RoPE'd at position l (Neox rotate-half), values copied
  - scatter back to paged layout

Sharding: core c = 2*b + g handles sequence b, heads [8g, 8g+8).

The host does all layout work so the device sees DMA-friendly shapes:
  - k source is pre-gathered (shift applied) and permuted to [128, 32*1024]:
    row r, block t = out position 128t+r, 8 heads x 128 dim. Every DMA row is
    16KB contiguous -> descriptors spread over all 16 SDMA engines.
  - v is a pure copy: contiguous old-page span + new tokens, moved DRAM->DRAM.
  - cos/sin tables [128, 4096] in the same position-permuted layout.
Device: 8 x [128,4096] k tiles -> RoPE (3 tensor_mul + 1 tensor_add on DVE,
tables broadcast over heads via stride-0 APs) -> store; v handled by gpsimd
DRAM->DRAM copies on an independent queue.
"""

import os
import numpy as np

KV_LEN = 4096
PAGE_SIZE = 16
SHIFT = 16
BSZ = 4
SEQ_LEN = 512
HEADS = 16
HEAD_DIM = 128
PAGES = 1024
PAGES_PER_SEQ = 256
ROPE_THETA = 10000.0

N_CORES = 8
HG = HEADS // 2                               # heads per core = 8
POS_SHIFT = SHIFT + SEQ_LEN                   # 528
KEEP = KV_LEN - POS_SHIFT                     # 3568 positions from old cache
TOK_END = KV_LEN - SHIFT                      # 4080
PAGE_SHIFT = POS_SHIFT // PAGE_SIZE           # 33
OLD_PAGES = PAGES_PER_SEQ - PAGE_SHIFT        # 223
FREE = HG * HEAD_DIM                          # 1024
NT = KV_LEN // 128                            # 32 position tiles
KW = NT * FREE                                # 32768 = k plane width per row
TILE_W = 4096                                 # free width per worked tile
NTILES = KW // TILE_W                         # 8
SUBT = TILE_W // FREE                         # 4 position-tiles per worked tile

_CACHE = {}


def _inv_freq():
    try:
        import jax
        import jax.numpy as jnp

        cpu = jax.devices("cpu")[0]
        with jax.default_device(cpu):
            f = 1.0 / (
                ROPE_THETA
                ** (jnp.arange(0, HEAD_DIM, 2, dtype=jnp.float32) / HEAD_DIM)
            )
            return np.asarray(f, dtype=np.float32)
    except Exception:
        return (
            np.float32(1.0)
            / (
                np.float32(ROPE_THETA)
                ** (np.arange(0, HEAD_DIM, 2, dtype=np.float32) / np.float32(HEAD_DIM))
            )
        ).astype(np.float32)


def _rope_factor_tables():
    """Factor tables for on-device construction of the [128,4096] cos/sin
    tables via the angle-addition identity. Out position l = 128t + r:
      cos(l*f) = cos(r*f)cos(128t*f) - sin(r*f)sin(128t*f)
      sigma_j * sin(l*f) = [sigma*sin(r*f)]cos(128t*f) + [sigma*cos(r*f)]sin(128t*f)
    where sigma_j = -1 for j<64, +1 for j>=64 (the rotate-half sign).
    Angles are exact (f64) multiples of the reference's f32 inv_freq."""
    f64 = _inv_freq().astype(np.float64)                       # [64]
    sigma = np.concatenate([-np.ones(64), np.ones(64)])

    r = np.arange(128, dtype=np.float64)
    ang_r = r[:, None] * f64[None, :]                          # [128,64]
    c0 = np.concatenate([np.cos(ang_r)] * 2, axis=1)           # [128,128]
    s0 = np.concatenate([np.sin(ang_r)] * 2, axis=1)

    t = np.arange(NT, dtype=np.float64) * 128.0
    ang_t = t[:, None] * f64[None, :]                          # [32,64]
    ct = np.concatenate([np.cos(ang_t)] * 2, axis=1)           # [32,128]
    st = np.concatenate([np.sin(ang_t)] * 2, axis=1)

    return {
        "row_c": np.ascontiguousarray(c0, dtype=np.float32),
        "row_s": np.ascontiguousarray(s0, dtype=np.float32),
        "row_ss": np.ascontiguousarray(sigma * s0, dtype=np.float32),
        "row_cs": np.ascontiguousarray(sigma * c0, dtype=np.float32),
        "col_c": np.ascontiguousarray(ct.reshape(1, KV_LEN), dtype=np.float32),
        "col_s": np.ascontiguousarray(st.reshape(1, KV_LEN), dtype=np.float32),
    }


def _build_program():
    from contextlib import ExitStack

    import concourse.bacc as bacc
    import concourse.tile as tile
    import concourse.mybir as mybir

    f32 = mybir.dt.float32
    nc = bacc.Bacc(
        "TRN2", target_bir_lowering=False, debug=False, enable_asserts=False
    )

    srck = nc.dram_tensor("srck", [128, KW], f32, kind="ExternalInput").ap()
    oldv = nc.dram_tensor("oldv", [OLD_PAGES, PAGE_SIZE, HG, HEAD_DIM], f32, kind="ExternalInput").ap()
    newv = nc.dram_tensor("newv", [SEQ_LEN, HG, HEAD_DIM], f32, kind="ExternalInput").ap()
    row_c = nc.dram_tensor("row_c", [128, 128], f32, kind="ExternalInput").ap()
    row_s = nc.dram_tensor("row_s", [128, 128], f32, kind="ExternalInput").ap()
    row_ss = nc.dram_tensor("row_ss", [128, 128], f32, kind="ExternalInput").ap()
    row_cs = nc.dram_tensor("row_cs", [128, 128], f32, kind="ExternalInput").ap()
    col_c = nc.dram_tensor("col_c", [1, KV_LEN], f32, kind="ExternalInput").ap()
    col_s = nc.dram_tensor("col_s", [1, KV_LEN], f32, kind="ExternalInput").ap()
    outk = nc.dram_tensor("out_k", [128, KW], f32, kind="ExternalOutput").ap()
    outv = nc.dram_tensor("out_v", [PAGES_PER_SEQ, PAGE_SIZE, HG, HEAD_DIM], f32, kind="ExternalOutput").ap()

    with tile.TileContext(nc) as tc:
        with ExitStack() as ctx:
            tabs = ctx.enter_context(tc.tile_pool(name="tables", bufs=1))
            cos_sb = tabs.tile([128, KV_LEN], f32)
            sin_sb = tabs.tile([128, KV_LEN], f32)

            # build the cos / signed-sin tables on device from the factor
            # tables instead of streaming 4MB from HBM
            fac = ctx.enter_context(tc.tile_pool(name="fac", bufs=1))
            rc = fac.tile([128, 128], f32, tag="rc")
            rs = fac.tile([128, 128], f32, tag="rs")
            rss = fac.tile([128, 128], f32, tag="rss")
            rcs = fac.tile([128, 128], f32, tag="rcs")
            cc = fac.tile([1, KV_LEN], f32, tag="cc")
            cs_ = fac.tile([1, KV_LEN], f32, tag="cs")
            nc.sync.dma_start(rc[:], row_c)
            nc.sync.dma_start(rs[:], row_s)
            nc.sync.dma_start(rss[:], row_ss)
            nc.sync.dma_start(rcs[:], row_cs)
            nc.sync.dma_start(cc[:], col_c)
            nc.sync.dma_start(cs_[:], col_s)

            t1p = ctx.enter_context(tc.tile_pool(name="t1", bufs=1))
            t2p = ctx.enter_context(tc.tile_pool(name="t2", bufs=1))
            outp = ctx.enter_context(tc.tile_pool(name="o", bufs=3))
            w1 = t1p.tile([128, KV_LEN], f32, tag="w1")
            w2 = t2p.tile([128, KV_LEN], f32, tag="w2")

            # replicate the column factors across all partitions (GpSimd)
            nc.gpsimd.partition_broadcast(w1[:], cc[:])
            nc.gpsimd.partition_broadcast(w2[:], cs_[:])

            def tj(ap):  # [128, KV_LEN] -> [128, NT, 128]
                return ap.rearrange("p (t j) -> p t j", j=128)

            def row_b(tile_):  # [128,128] broadcast along t (stride-0 free dim)
                return tile_[:].unsqueeze(1).broadcast_to([128, NT, 128])

            ta = outp.tile([128, KV_LEN], f32, tag="O")
            tb = outp.tile([128, KV_LEN], f32, tag="O")
            nc.vector.tensor_mul(tj(ta[:]), row_b(rc), tj(w1[:]))
            nc.vector.tensor_mul(tj(tb[:]), row_b(rs), tj(w2[:]))
            nc.vector.tensor_sub(cos_sb[:], ta[:], tb[:])
            tc_ = outp.tile([128, KV_LEN], f32, tag="O")
            td = outp.tile([128, KV_LEN], f32, tag="O")
            nc.vector.tensor_mul(tj(tc_[:]), row_b(rss), tj(w1[:]))
            nc.vector.tensor_mul(tj(td[:]), row_b(rcs), tj(w2[:]))
            nc.vector.tensor_add(sin_sb[:], tc_[:], td[:])

            # v path: contiguous DRAM->DRAM copies on the gpsimd (SWDGE)
            # queue, independent of both HWDGE rings.
            # out pages 0..222 <- old pages 33..255 (= oldv[0:223])
            nc.gpsimd.dma_start(
                outv[0:OLD_PAGES], oldv[:], max_dma_last_dim=16384
            )
            # out pages 223..254 <- new v tokens
            nc.gpsimd.dma_start(
                outv[OLD_PAGES : OLD_PAGES + SEQ_LEN // PAGE_SIZE],
                newv[:],
                max_dma_last_dim=16384,
            )
            # out page 255 <- old page 255 (= oldv[222])
            nc.gpsimd.dma_start(outv[255], oldv[OLD_PAGES - 1])

            xp = ctx.enter_context(tc.tile_pool(name="x", bufs=3))

            for i in range(NTILES):
                X = xp.tile([128, TILE_W], f32)
                nc.sync.dma_start(X[:], srck[:, i * TILE_W : (i + 1) * TILE_W])

                Xr = X[:].rearrange("p (s h d) -> p s h d", h=HG, d=HEAD_DIM)
                cs = cos_sb[:, i * SUBT * 128 : (i + 1) * SUBT * 128].rearrange(
                    "p (s j) -> p s j", j=128
                )
                sn = sin_sb[:, i * SUBT * 128 : (i + 1) * SUBT * 128].rearrange(
                    "p (s j) -> p s j", j=128
                )
                cos_b = cs.unsqueeze(2).broadcast_to([128, SUBT, HG, 128])
                sin_a = sn[:, :, 0:64].unsqueeze(2).broadcast_to([128, SUBT, HG, 64])
                sin_b = sn[:, :, 64:128].unsqueeze(2).broadcast_to([128, SUBT, HG, 64])

                T1 = t1p.tile([128, TILE_W], f32, tag="w1")
                T2 = t2p.tile([128, TILE_W], f32, tag="w2")
                T1r = T1[:].rearrange("p (s h d) -> p s h d", h=HG, d=HEAD_DIM)
                T2r = T2[:].rearrange("p (s h d) -> p s h d", h=HG, d=HEAD_DIM)

                nc.vector.tensor_mul(T1r, Xr, cos_b)
                nc.vector.tensor_mul(T2r[:, :, :, 0:64], Xr[:, :, :, 64:128], sin_a)
                nc.vector.tensor_mul(T2r[:, :, :, 64:128], Xr[:, :, :, 0:64], sin_b)

                O = outp.tile([128, TILE_W], f32, tag="O")
                nc.vector.tensor_add(O[:], T1[:], T2[:])
                nc.scalar.dma_start(outk[:, i * TILE_W : (i + 1) * TILE_W], O[:])

    nc.compile()
    return nc


def _get_program():
    if "nc" not in _CACHE:
        _CACHE["nc"] = _build_program()
    return _CACHE["nc"]


def kernel(
    k,
    v,
    draft_cache,
    kv_page_indices,
    bsz=BSZ,
    context_len=KV_LEN,
    seq_len=SEQ_LEN,
    n_local_heads=HEADS,
    head_dim=HEAD_DIM,
):
    from concourse.bass_utils import run_bass_kernel_spmd

    k = np.asarray(k, dtype=np.float32)
    v = np.asarray(v, dtype=np.float32)
    draft_cache = np.asarray(draft_cache, dtype=np.float32)
    kv_page_indices = np.asarray(kv_page_indices)

    pages = kv_page_indices.reshape(BSZ, PAGES_PER_SEQ)
    identity = bool(
        np.array_equal(kv_page_indices, np.arange(PAGES, dtype=kv_page_indices.dtype))
    )

    kb = k.reshape(BSZ, SEQ_LEN, HEADS, HEAD_DIM)
    vb = v.reshape(BSZ, SEQ_LEN, HEADS, HEAD_DIM)

    if "tables" not in _CACHE:
        _CACHE["tables"] = _rope_factor_tables()
    fac = _CACHE["tables"]

    in_maps = []
    for c in range(N_CORES):
        b, g = divmod(c, 2)
        if identity:
            cache_b = draft_cache[b * PAGES_PER_SEQ : (b + 1) * PAGES_PER_SEQ]
        else:
            cache_b = draft_cache[pages[b]]
        hsl = slice(g * HG, (g + 1) * HG)

        # k source with the shift applied, then position-permuted
        oldk = cache_b[:, 0, :, hsl, :].reshape(KV_LEN, HG, HEAD_DIM)
        srck = np.empty((KV_LEN, HG, HEAD_DIM), np.float32)
        srck[0:KEEP] = oldk[POS_SHIFT:KV_LEN]
        srck[KEEP:TOK_END] = kb[b, :, hsl, :]
        srck[TOK_END:] = oldk[TOK_END:]
        srck_perm = np.ascontiguousarray(
            srck.reshape(NT, 128, FREE).transpose(1, 0, 2).reshape(128, KW)
        )

        oldv_c = np.ascontiguousarray(cache_b[PAGE_SHIFT:, 1, :, hsl, :])
        newv_c = np.ascontiguousarray(vb[b, :, hsl, :])

        in_maps.append(
            {
                "srck": srck_perm,
                "oldv": oldv_c,
                "newv": newv_c,
                **fac,
            }
        )

    nc = _get_program()
    trace = bool(int(os.environ.get("KVC_TRACE", "0")))
    res = run_bass_kernel_spmd(
        nc,
        in_maps,
        list(range(N_CORES)),
        trace=trace,
        trace_cores=list(range(N_CORES)) if trace else None,
    )
    _CACHE["last_results"] = res

    kv_pages = np.empty((PAGES, 2, PAGE_SIZE, HEADS, HEAD_DIM), dtype=np.float32)
    for c in range(N_CORES):
        b, g = divmod(c, 2)
        hsl = slice(g * HG, (g + 1) * HG)
        psl = slice(b * PAGES_PER_SEQ, (b + 1) * PAGES_PER_SEQ)
        outk = res.results[c]["out_k"]
        klin = (
            outk.reshape(128, NT, HG, HEAD_DIM)
            .transpose(1, 0, 2, 3)
            .reshape(PAGES_PER_SEQ, PAGE_SIZE, HG, HEAD_DIM)
        )
        kv_pages[psl, 0, :, hsl, :] = klin
        kv_pages[psl, 1, :, hsl, :] = res.results[c]["out_v"]

    if identity:
        return kv_pages
    rotated = draft_cache.copy()
    rotated[kv_page_indices] = kv_pages
    return rotated
